# revision 1
# baseline (speedup 1.0000x reference)
"""Bass/Trainium2 kernel for nn_EnergyModel (3-layer GAT + MLP head).

Sharding: data-parallel over batch B=32 across 8 NeuronCores (4 graphs/core),
GAT/MLP params replicated.

Per-graph algorithm (per core, 3 GAT layers):
  - atomsT [c_in, 256] kept transposed (c on partitions).
  - h   = atoms @ W         -> PE, out [i, (r,c)] psum -> sbuf  (agg lhsT)
  - hT  blocks [c, i] per r -> PE (lhsT = W r-slice)            (srcdst rhs)
  - src/dst = a . h         -> PE (lhsT = aTI[:, r, :] [c, 2])
  - S[(r,j)-block, i] = dst_j + src_i  -> PE rank-2 matmul with augmented
    [dst|ones] x [ones|src] tiles.
  - additive mask A = (bond-1)*3e38 built in natural layout (one DVE pass per
    graph), transposed to [(r,j), i] blocks via bf16 xbar DMA transposes.
  - T = S + A (DVE/GPSIMD), L = max(T, 0.2T) (DVE/GPSIMD), Et = Exp(L) (ACT).
  - out^T[c, i] = sum_b h_b^T Et_b (PE, k=1280 accumulation)
  - Z[1, i] = ones^T Et (PE), rz = 1/Z, broadcast by rank-1 matmul,
    atomsT_next = leaky(out^T) * rz  (leaky commutes with positive scale).
  - layer 3: y_feats from mean/max over i; MLP head batched over 4 graphs.
"""

import sys
from contextlib import ExitStack

if "/opt/trn_rl_repo" not in sys.path:
    sys.path.insert(0, "/opt/trn_rl_repo")

import numpy as np

B, N, CIN, C, R, XD = 32, 256, 64, 128, 5, 1024
NCORE = 8
NG = B // NCORE  # graphs per core
NRC = R * C      # 640
H1 = 256         # MLP hidden 1
H2 = 32          # MLP hidden 2
ZDIM = 2 * C + XD  # 1280

_BUILD_CACHE = {}
POOL_CFG = {'gpool': 2, 'spool': 2, 'ps_s': 2, 'ps_sm': 4, 'et': 2, 'tlb': 2, 'h': 2, 'ht': 2}


def build(n_graphs=NG, with_bias=True, repeat=1):
    key = (n_graphs, with_bias, repeat)
    if key in _BUILD_CACHE:
        return _BUILD_CACHE[key]

    import concourse.bass as bass
    from concourse import bacc
    import concourse.tile as tile
    import concourse.mybir as mybir
    from concourse.masks import make_identity

    f32 = mybir.dt.float32
    f32r = mybir.dt.float32r
    bf16 = mybir.dt.bfloat16
    i32 = mybir.dt.int32
    AF = mybir.ActivationFunctionType
    OP = mybir.AluOpType

    def mm(out, lhsT, rhs, **kw):
        nc.tensor.matmul(out, lhsT, rhs, **kw)

    nc = bacc.Bacc("TRN2", target_bir_lowering=False)
    ng = n_graphs

    atoms_d = nc.dram_tensor("y_atoms", [ng, N, CIN], f32, kind="ExternalInput")
    bonds_d = nc.dram_tensor("y_bonds", [ng, N, N, R], i32, kind="ExternalInput")
    x_d = nc.dram_tensor("x", [ng, XD], f32, kind="ExternalInput")
    W_d = [
        nc.dram_tensor("W1", [CIN, NRC], f32, kind="ExternalInput"),
        nc.dram_tensor("W2", [C, NRC], f32, kind="ExternalInput"),
        nc.dram_tensor("W3", [C, NRC], f32, kind="ExternalInput"),
    ]
    a_d = [
        nc.dram_tensor(f"a{i}", [R, 2 * C], f32, kind="ExternalInput")
        for i in (1, 2, 3)
    ]
    We1_d = nc.dram_tensor("We1", [ZDIM, H1], f32, kind="ExternalInput")
    We2_d = nc.dram_tensor("We2", [H1, H2], f32, kind="ExternalInput")
    We3_d = nc.dram_tensor("We3", [H2, 1], f32, kind="ExternalInput")
    if with_bias:
        b_d = [
            nc.dram_tensor(f"b{i}", [1, NRC], f32, kind="ExternalInput")
            for i in (1, 2, 3)
        ]
        be1_d = nc.dram_tensor("be1", [1, H1], f32, kind="ExternalInput")
        be2_d = nc.dram_tensor("be2", [1, H2], f32, kind="ExternalInput")
        be3_d = nc.dram_tensor("be3", [1, 1], f32, kind="ExternalInput")
    out_d = nc.dram_tensor("out", [ng, 1], f32, kind="ExternalOutput")

    with tile.TileContext(nc) as tc, ExitStack() as ctx:
        const = ctx.enter_context(tc.tile_pool(name="const", bufs=1))
        gpool = ctx.enter_context(tc.tile_pool(name="gpool", bufs=POOL_CFG["gpool"]))
        gpool3 = ctx.enter_context(tc.tile_pool(name="gpool3", bufs=POOL_CFG["et"]))
        spool = ctx.enter_context(tc.tile_pool(name="spool", bufs=POOL_CFG["spool"]))
        ps_s = ctx.enter_context(tc.tile_pool(name="ps_s", bufs=POOL_CFG["ps_s"], space="PSUM"))
        ps_sm = ctx.enter_context(tc.tile_pool(name="ps_sm", bufs=POOL_CFG["ps_sm"], space="PSUM"))

        # ---------------- constants ----------------
        ident = const.tile([128, 128], f32)
        make_identity(nc, ident[:])
        onesf = const.tile([128, 1], f32)
        nc.vector.memset(onesf[:], 1.0)
        ones_col = const.tile([128, 1], f32r)
        nc.vector.tensor_copy(ones_col[:], onesf[:])
        onesrf = const.tile([1, 256], f32)
        nc.vector.memset(onesrf[:], 1.0)
        ones_row = const.tile([1, 256], f32r)
        nc.vector.tensor_copy(ones_row[:], onesrf[:])

        W_sb = []
        for li in range(3):
            cin = CIN if li == 0 else C
            w_raw = spool.tile([cin, NRC], f32, tag="w_raw")
            nc.sync.dma_start(w_raw[:], W_d[li][:])
            w = const.tile([cin, NRC], f32r, tag=f"W{li}")
            nc.vector.tensor_copy(w[:], w_raw[:])
            W_sb.append(w)

        # Asel[l]: [c, r, m] block-diagonal src/dst selector: column m=r of
        # k-chunk r holds the src half a[r, c]; column m=R+r the dst half
        # a[r, C+c]; other columns zero. One accumulated matmul over the 5
        # k-chunks then yields sd[m, i].
        Asel_sb = []
        for li in range(3):
            # aT[c, r, s] = a[r, s*C + c] via element-strided (one-time) DMA
            aT = spool.tile([C, R, 2], f32, tag="a_t")
            nc.sync.dma_start(aT[:], a_d[li].rearrange("r (s c) -> c r s", s=2))
            Asel = const.tile([C, R, 2 * R], f32r, tag=f"asel{li}")
            nc.vector.memset(Asel[:].bitcast(f32), 0.0)
            for s in range(2):
                for r in range(R):
                    nc.scalar.activation(
                        Asel[:, r, s * R + r:s * R + r + 1], aT[:, r, s:s + 1],
                        AF.Copy,
                    )
            Asel_sb.append(Asel)

        We1_raw = const.tile([128, 10, H1], f32)
        nc.sync.dma_start(We1_raw[:],
                          We1_d.rearrange("(kb p) n -> p kb n", p=128))
        We1_sb = const.tile([128, 10, H1], f32r)
        nc.vector.tensor_copy(We1_sb[:], We1_raw[:])
        We2_sb = const.tile([128, 2, H2], f32)
        nc.sync.dma_start(We2_sb[:],
                          We2_d.rearrange("(kb p) n -> p kb n", p=128))
        We3_sb = const.tile([H2, 1], f32)
        nc.sync.dma_start(We3_sb[:], We3_d[:])

        if with_bias:
            b_row = []
            for li in range(3):
                braw = spool.tile([1, NRC], f32, tag="braw")
                nc.sync.dma_start(braw[:], b_d[li][:])
                br = const.tile([1, NRC], f32r, tag=f"brow{li}")
                nc.vector.tensor_copy(br[:], braw[:])
                b_row.append(br)
            beraw = spool.tile([1, H1], f32, tag="beraw")
            nc.sync.dma_start(beraw[:], be1_d[:])
            be1_row = const.tile([1, H1], f32r)
            nc.vector.tensor_copy(be1_row[:], beraw[:])
            be2_row = const.tile([1, H2], f32)
            nc.sync.dma_start(be2_row[:], be2_d[:])
            be3_row = const.tile([1, 1], f32)
            nc.sync.dma_start(be3_row[:], be3_d[:])

        # MLP lhsT staging: z^T chunks [128, kb, g]; kb 0..7 = x, 8 = mean, 9 = max
        zT = const.tile([128, 10, ng], f32r)

        # ---------------- per graph (repeat>1 only for benchmarking) ----
        for _rep in range(repeat):
         for g in range(ng):
            # -- bonds -> additive mask (natural layout, r-major free order) --
            # A_T[j', b, ib, i'] (block b = r*2+jh); per-iblock staging tiles
            A_T = gpool.tile([128, 10, 2, 128], bf16, tag="at")
            for ib in range(2):
                bonds_sb = gpool.tile([128, N * R], i32, tag="bonds")
                nc.sync.dma_start(
                    bonds_sb[:],
                    bonds_d[g, ib * 128:(ib + 1) * 128].rearrange("p j r -> p (j r)"),
                )
                # A_bf[p, r, jh, j'] = (bond[i=p+128*ib, j=jh*128+j', r] - 1)*3e38
                A_bf = gpool.tile([128, R, 2, 128], bf16, tag="abf")
                nc.vector.tensor_scalar(
                    A_bf[:],
                    bonds_sb.rearrange("p (jh j r) -> p r jh j", jh=2, j=128, r=R),
                    1,
                    3.0e38,
                    op0=OP.subtract,
                    op1=OP.mult,
                )
                for b in range(10):
                    r, jh = b // 2, b % 2
                    nc.sync.dma_start_transpose(
                        A_T[:, b, ib, :], A_bf[:, r, jh, :]
                    )

            # per-graph aug tiles: dstP p0=dst/p1=ones, srcP p0=ones/p1=src
            # (double-buffered across layers; ones rows set once per graph)
            dstP_g = []
            srcP_g = []
            for db in range(2):
                dP = gpool.tile([2, R, 256], f32r, tag=f"dstp{db}")
                sP = gpool.tile([2, R, 256], f32r, tag=f"srcp{db}")
                nc.gpsimd.memset(dP[:].bitcast(f32), 1.0)
                nc.gpsimd.memset(sP[:].bitcast(f32), 1.0)
                dstP_g.append(dP)
                srcP_g.append(sP)

            # -- atoms transpose --
            at_nat = spool.tile([128, 2, CIN], f32, tag="atnat")
            for ib in range(2):
                nc.sync.dma_start(at_nat[:, ib, :], atoms_d[g, ib * 128:(ib + 1) * 128, :])
            atT_ps = ps_sm.tile([CIN, 2, 128], f32, tag="sm")
            for ib in range(2):
                nc.tensor.matmul(
                    atT_ps[:, ib, :], at_nat[:, ib, :], ident[:],
                    is_transpose=True, start=True, stop=True,
                )
            atoms_cur = gpool.tile([CIN, 256], f32r, tag="atoms0")
            nc.vector.tensor_copy(
                atoms_cur[:], atT_ps.rearrange("c a b -> c (a b)")
            )

            # -- x staging for MLP (stage f32, round-copy into f32r zT) --
            x_stage = spool.tile([128, 8], f32, tag="xstage")
            nc.sync.dma_start(x_stage[:], x_d[g].rearrange("(f p) -> p f", p=128))
            nc.vector.tensor_copy(zT[:, 0:8, g:g + 1].rearrange("p a b -> p (a b)"),
                                  x_stage[:])

            # ---------------- GAT layers ----------------
            for li in range(3):
                W = W_sb[li]
                # h = atoms @ W (+b): out [i, (r,c)] in two n-chunks per i-block
                h_sb = gpool3.tile([128, 2, NRC], f32r, tag="h")
                for ib in range(2):
                    hA = ps_sm.tile([128, 384], f32, tag="sm")
                    hB = ps_sm.tile([128, 256], f32, tag="sm")
                    lt = atoms_cur[:, ib * 128:(ib + 1) * 128]
                    mm(hA[:], lt, W[:, 0:384], start=True, stop=not with_bias)
                    mm(hB[:], lt, W[:, 384:NRC], start=True, stop=not with_bias)
                    if with_bias:
                        mm(hA[:], ones_row[:, :128], b_row[li][:, 0:384],
                           start=False, stop=True)
                        mm(hB[:], ones_row[:, :128], b_row[li][:, 384:NRC],
                           start=False, stop=True)
                    if ib == 0:
                        nc.scalar.activation(h_sb[:, ib, 0:384], hA[:], AF.Copy)
                        nc.vector.tensor_copy(h_sb[:, ib, 384:NRC], hB[:])
                    else:
                        nc.vector.tensor_copy(h_sb[:, ib, 0:384], hA[:])
                        nc.scalar.activation(h_sb[:, ib, 384:NRC], hB[:], AF.Copy)

                # hT blocks: [c, i] per r (lhsT = W r-slice); 2-bank psum tiles
                hT_sb = gpool3.tile([128, R, 256], f32r, tag="ht")
                for rp in range(3):
                    rr = (2, 2, 1)[rp]
                    r0 = 2 * rp
                    hT_ps = ps_sm.tile([128, 2, 256], f32, tag="sm")
                    for dr in range(rr):
                        r = r0 + dr
                        mm(hT_ps[:, dr, :], W[:, r * 128:(r + 1) * 128],
                           atoms_cur[:], start=True, stop=not with_bias)
                        if with_bias:
                            mm(hT_ps[:, dr, :],
                               b_row[li][:, r * 128:(r + 1) * 128],
                               ones_row[:], start=False, stop=True)
                    nc.vector.tensor_copy(
                        hT_sb[:, r0:r0 + rr, :].rearrange("p a b -> p (a b)"),
                        hT_ps[:, 0:rr, :].rearrange("p a b -> p (a b)"),
                    )

                # src/dst: sd_ps[2r+s, i] via accumulated block-diag matmul
                sd_ps = ps_sm.tile([2 * R, 256], f32, tag="sm")
                for r in range(R):
                    mm(sd_ps[:], Asel_sb[li][:, r, :], hT_sb[:, r, :],
                       start=(r == 0), stop=(r == R - 1))
                # evict sd rows to sbuf at base 32 (src rows 32-36, dst 37-41)
                sd_sb = spool.tile([42, 256], f32r, tag="sdsb")
                nc.vector.tensor_copy(sd_sb[32:42, :], sd_ps[:])
                # gather into the per-graph aug tiles (ones rows pre-set)
                dstP = dstP_g[li % 2]
                srcP = srcP_g[li % 2]
                nc.sync.dma_start(dstP[0:1], sd_sb[32 + R:32 + 2 * R, :])
                nc.sync.dma_start(srcP[1:2], sd_sb[32:32 + R, :])

                # S blocks -> mask add -> leaky -> exp, in 2 chunks of 5 blocks
                Et = gpool3.tile([128, 10, 256], f32r, tag="et")
                b0 = 0
                for nb in (4, 4, 2):
                    T_sb = gpool3.tile([128, 4, 256], f32, tag="tsb")
                    L_sb = gpool3.tile([128, 4, 256], f32, tag="lsb")
                    S_ps = ps_s.tile([128, 4, 256], f32, tag="sps")
                    for k in range(nb):
                        b = b0 + k
                        r, jh = b // 2, b % 2
                        mm(S_ps[:, k, :],
                           dstP[:, r, jh * 128:(jh + 1) * 128],
                           srcP[:, r, :],
                           start=True, stop=True)
                    # T = S + A
                    nc.vector.tensor_tensor(
                        T_sb[:, 0:nb].rearrange("p a b -> p (a b)"),
                        S_ps[:, 0:nb].rearrange("p a b -> p (a b)"),
                        A_T[:, b0:b0 + nb].rearrange("p a b c -> p (a b c)"),
                        op=OP.add,
                    )
                    # L = leaky(T) on ACT (Prelu alpha=0.2; HW-exact)
                    nc.scalar.activation(
                        L_sb[:, 0:nb].rearrange("p a b -> p (a b)"),
                        T_sb[:, 0:nb].rearrange("p a b -> p (a b)"),
                        AF.Prelu, alpha=0.2,
                    )
                    # Et = exp(L)
                    nc.scalar.activation(
                        Et[:, b0:b0 + nb].rearrange("p a b -> p (a b)"),
                        L_sb[:, 0:nb].rearrange("p a b -> p (a b)"),
                        AF.Exp,
                    )
                    b0 += nb

                # aggregation out^T = sum_b h_b^T @ Et_b ; Z = sum_b ones^T @ Et_b
                o_ps = ps_sm.tile([128, 256], f32, tag="sm")
                for b in range(10):
                    r, jh = b // 2, b % 2
                    mm(o_ps[:], h_sb[:, jh, r * 128:(r + 1) * 128],
                       Et[:, b, :], start=(b == 0), stop=(b == 9))
                z_ps = ps_sm.tile([1, 256], f32, tag="sm")
                for b in range(10):
                    mm(z_ps[:], ones_col[:],
                       Et[:, b, :], start=(b == 0), stop=(b == 9))

                # normalize (+ inter-layer leaky)
                rz_sb = spool.tile([1, 256], f32r, tag="rz")
                with nc.allow_low_precision(reason="f32r recip, 2^-12 rounding ok"):
                    nc.vector.reciprocal(rz_sb[:], z_ps[:])
                rzb_ps = ps_sm.tile([128, 256], f32, tag="sm")
                mm(rzb_ps[:], ones_row[:, :128], rz_sb[:], start=True, stop=True)
                O_sb = spool.tile([128, 256], f32, tag="osb")
                if li < 2:
                    nc.scalar.activation(O_sb[:], o_ps[:], AF.Prelu, alpha=0.2)
                else:
                    nc.scalar.activation(O_sb[:], o_ps[:], AF.Copy)
                nxt = gpool.tile([C, 256], f32r, tag=f"atoms{li + 1}")
                nc.vector.tensor_tensor(nxt[:], O_sb[:], rzb_ps[:], op=OP.mult)
                atoms_cur = nxt

            # y_feats: mean/max over atoms (free dim of h3T [c, i])
            h3T = atoms_cur
            mean_raw = spool.tile([128, 1], f32, tag="mean")
            nc.vector.tensor_reduce(mean_raw[:], h3T[:], axis=mybir.AxisListType.X,
                                    op=OP.add)
            nc.vector.tensor_scalar(zT[:, 8, g:g + 1], mean_raw[:], 1.0 / N, None,
                                    op0=OP.mult)
            nc.vector.tensor_reduce(zT[:, 9, g:g + 1], h3T[:], axis=mybir.AxisListType.X,
                                    op=OP.max)

         # ---------------- MLP head (batched over graphs) ---------------
         zz_ps = ps_sm.tile([ng, H1], f32, tag="sm")
         for kb in range(10):
            mm(zz_ps[:], zT[:, kb, :], We1_sb[:, kb, :],
               start=(kb == 0), stop=(kb == 9) and not with_bias)
         if with_bias:
            mm(zz_ps[:], ones_row[:, :ng], be1_row[:], start=False, stop=True)
         zzl = spool.tile([ng, H1], f32, tag="zzl")
         nc.scalar.activation(zzl[:], zz_ps[:], AF.Prelu, alpha=0.2)
         zzT_ps = ps_sm.tile([128, 2, ng], f32, tag="sm")
         for hh in range(2):
            nc.tensor.matmul(zzT_ps[:, hh, :], zzl[:, hh * 128:(hh + 1) * 128],
                             ident[:ng, :ng], is_transpose=True,
                             start=True, stop=True)
         zzT_sb = spool.tile([128, 2, ng], f32, tag="zzt")
         nc.vector.tensor_copy(zzT_sb[:], zzT_ps[:])

         z2_ps = ps_sm.tile([ng, H2], f32, tag="sm")
         for hh in range(2):
            nc.tensor.matmul(z2_ps[:], zzT_sb[:, hh, :], We2_sb[:, hh, :],
                             start=(hh == 0), stop=(hh == 1) and not with_bias)
         if with_bias:
            nc.tensor.matmul(z2_ps[:], onesrf[:, :ng], be2_row[:],
                             start=False, stop=True)
         z2l = spool.tile([ng, H2], f32, tag="z2l")
         nc.scalar.activation(z2l[:], z2_ps[:], AF.Prelu, alpha=0.2)
         z2T_ps = ps_sm.tile([H2, ng], f32, tag="sm")
         nc.tensor.matmul(z2T_ps[:], z2l[:], ident[:ng, :ng], is_transpose=True,
                         start=True, stop=True)
         z2T_sb = spool.tile([H2, ng], f32, tag="z2t")
         nc.vector.tensor_copy(z2T_sb[:], z2T_ps[:])

         y_ps = ps_sm.tile([ng, 1], f32, tag="sm")
         nc.tensor.matmul(y_ps[:], z2T_sb[:], We3_sb[:], start=True,
                         stop=not with_bias)
         if with_bias:
            nc.tensor.matmul(y_ps[:], onesrf[:, :ng], be3_row[:],
                             start=False, stop=True)
         y_sb = spool.tile([ng, 1], f32, tag="y")
         nc.vector.tensor_copy(y_sb[:], y_ps[:])
         nc.sync.dma_start(out_d[:], y_sb[:])

    nc.compile()
    _BUILD_CACHE[key] = nc
    return nc


_PARAM_KEYS = ("W1", "W2", "W3", "a1", "a2", "a3", "We1", "We2", "We3")
_BIAS_KEYS = ("b1", "b2", "b3", "be1", "be2", "be3")


def _shard_inputs(inputs, with_bias, n_cores, ng):
    per_core = []
    for c in range(n_cores):
        s = slice(c * ng, (c + 1) * ng)
        m = {
            "y_atoms": np.ascontiguousarray(inputs["y_atoms"][s], np.float32),
            "y_bonds": np.ascontiguousarray(inputs["y_bonds"][s], np.int32),
            "x": np.ascontiguousarray(inputs["x"][s], np.float32),
        }
        for k in _PARAM_KEYS:
            m[k] = np.ascontiguousarray(inputs[k], np.float32)
        if with_bias:
            for k in _BIAS_KEYS:
                m[k] = np.ascontiguousarray(np.asarray(inputs[k], np.float32).reshape(1, -1))
        per_core.append(m)
    return per_core


def _needs_bias(inputs):
    return any(np.abs(np.asarray(inputs[k])).max() > 0 for k in _BIAS_KEYS)


def kernel(**inputs):
    from concourse.bass_utils import run_bass_kernel_spmd

    with_bias = _needs_bias(inputs)
    nc = build(NG, with_bias)
    in_maps = _shard_inputs(inputs, with_bias, NCORE, NG)
    res = run_bass_kernel_spmd(nc, in_maps, core_ids=list(range(NCORE)))
    out = np.concatenate([r["out"] for r in res.results], axis=0)
    return np.ascontiguousarray(out, np.float32)



# revision 2
# speedup vs baseline: 1.5098x; 1.5098x over previous
"""Bass/Trainium2 kernel for nn_EnergyModel (3-layer GAT + MLP head).

Sharding: data-parallel over batch B=32 across 8 NeuronCores (4 graphs/core),
GAT/MLP params replicated.

Key design (v2):
  - Host pre-transposes bonds to [g, j', b=(2r+jh), i] so the attention mask
    loads directly in the transposed layout the PE aggregation needs — no
    on-device transposes (the v1 DMA-transpose path serialized the sync
    engine). Mask is MULTIPLICATIVE: Et = exp(prelu(S)) * bond.
  - Host folds W@a into Wsd [cin, 10] so src/dst logits come straight from
    atomsT in one matmul: sd = Wsd.T @ atomsT (rows 0-4 src_r, 5-9 dst_r).
    This removes the per-layer hT computation entirely.
  - S[j', i] per block (r, jh) via rank-2 augmented outer product
    [dst|ones]^T [ones|src] (f32r).
  - prelu (ACT) -> exp (ACT, bf16 out) -> mask-mult (DVE, bf16 2x mode).
  - aggregation out^T = sum_b h_b^T Et_b and Z = ones^T Et on PE in bf16
    (fast weight loads); h kept bf16, logit path kept f32r.
  - normalization / y_feats / MLP head as before.
"""

import sys
from contextlib import ExitStack

if "/opt/trn_rl_repo" not in sys.path:
    sys.path.insert(0, "/opt/trn_rl_repo")

import numpy as np

B, N, CIN, C, R, XD = 32, 256, 64, 128, 5, 1024
NCORE = 8
NG = B // NCORE  # graphs per core
NRC = R * C      # 640
NB = 2 * R       # 10 blocks b = 2r + jh
H1 = 256         # MLP hidden 1
H2 = 32          # MLP hidden 2
ZDIM = 2 * C + XD  # 1280

_BUILD_CACHE = {}


def build(n_graphs=NG, with_bias=True, repeat=1):
    key = (n_graphs, with_bias, repeat)
    if key in _BUILD_CACHE:
        return _BUILD_CACHE[key]

    import concourse.bass as bass
    from concourse import bacc
    import concourse.tile as tile
    import concourse.mybir as mybir
    from concourse.masks import make_identity

    f32 = mybir.dt.float32
    f32r = mybir.dt.float32r
    bf16 = mybir.dt.bfloat16
    i32 = mybir.dt.int32
    AF = mybir.ActivationFunctionType
    OP = mybir.AluOpType

    def mm(out, lhsT, rhs, **kw):
        nc.tensor.matmul(out, lhsT, rhs, **kw)

    nc = bacc.Bacc("TRN2", target_bir_lowering=False)
    ng = n_graphs

    atoms_d = nc.dram_tensor("y_atoms", [ng, N, CIN], f32, kind="ExternalInput")
    bondsT_d = nc.dram_tensor("bonds_t", [ng, 128, NB, N], i32, kind="ExternalInput")
    x_d = nc.dram_tensor("x", [ng, XD], f32, kind="ExternalInput")
    W_d = [
        nc.dram_tensor("W1", [CIN, NRC], f32, kind="ExternalInput"),
        nc.dram_tensor("W2", [C, NRC], f32, kind="ExternalInput"),
        nc.dram_tensor("W3", [C, NRC], f32, kind="ExternalInput"),
    ]
    Wsd_d = [
        nc.dram_tensor(f"Wsd{i}", [CIN if i == 1 else C, NB], f32,
                       kind="ExternalInput")
        for i in (1, 2, 3)
    ]
    We1_d = nc.dram_tensor("We1", [ZDIM, H1], f32, kind="ExternalInput")
    We2_d = nc.dram_tensor("We2", [H1, H2], f32, kind="ExternalInput")
    We3_d = nc.dram_tensor("We3", [H2, 1], f32, kind="ExternalInput")
    if with_bias:
        b_d = [
            nc.dram_tensor(f"b{i}", [1, NRC], f32, kind="ExternalInput")
            for i in (1, 2, 3)
        ]
        bsd_d = [
            nc.dram_tensor(f"bsd{i}", [1, NB], f32, kind="ExternalInput")
            for i in (1, 2, 3)
        ]
        be1_d = nc.dram_tensor("be1", [1, H1], f32, kind="ExternalInput")
        be2_d = nc.dram_tensor("be2", [1, H2], f32, kind="ExternalInput")
        be3_d = nc.dram_tensor("be3", [1, 1], f32, kind="ExternalInput")
    out_d = nc.dram_tensor("out", [ng, 1], f32, kind="ExternalOutput")

    with tile.TileContext(nc) as tc, ExitStack() as ctx:
        const = ctx.enter_context(tc.tile_pool(name="const", bufs=1))
        gpool = ctx.enter_context(tc.tile_pool(name="gpool", bufs=2))
        gpool3 = ctx.enter_context(tc.tile_pool(name="gpool3", bufs=2))
        spool = ctx.enter_context(tc.tile_pool(name="spool", bufs=2))
        ps_s = ctx.enter_context(tc.tile_pool(name="ps_s", bufs=2, space="PSUM"))
        ps_sm = ctx.enter_context(tc.tile_pool(name="ps_sm", bufs=4, space="PSUM"))

        # ---------------- constants ----------------
        ident = const.tile([128, 128], f32)
        make_identity(nc, ident[:])
        onesf = const.tile([128, 1], f32)
        nc.vector.memset(onesf[:], 1.0)
        ones_bf = const.tile([128, 1], bf16)
        nc.vector.memset(ones_bf[:], 1.0)
        onesrf = const.tile([1, 256], f32)
        nc.vector.memset(onesrf[:], 1.0)
        ones_row = const.tile([1, 256], f32r)
        nc.vector.tensor_copy(ones_row[:], onesrf[:])

        W_sb = []
        Wsd_sb = []
        for li in range(3):
            cin = CIN if li == 0 else C
            w_raw = spool.tile([cin, NRC], f32, tag="w_raw")
            nc.sync.dma_start(w_raw[:], W_d[li][:])
            w = const.tile([cin, NRC], f32r, tag=f"W{li}")
            nc.vector.tensor_copy(w[:], w_raw[:])
            W_sb.append(w)
            wsd_raw = spool.tile([cin, NB], f32, tag="wsd_raw")
            nc.sync.dma_start(wsd_raw[:], Wsd_d[li][:])
            wsd = const.tile([cin, NB], f32r, tag=f"Wsd{li}")
            nc.vector.tensor_copy(wsd[:], wsd_raw[:])
            Wsd_sb.append(wsd)

        We1_raw = const.tile([128, 10, H1], f32)
        nc.sync.dma_start(We1_raw[:],
                          We1_d.rearrange("(kb p) n -> p kb n", p=128))
        We1_sb = const.tile([128, 10, H1], f32r)
        nc.vector.tensor_copy(We1_sb[:], We1_raw[:])
        We2_sb = const.tile([128, 2, H2], f32)
        nc.sync.dma_start(We2_sb[:],
                          We2_d.rearrange("(kb p) n -> p kb n", p=128))
        We3_sb = const.tile([H2, 1], f32)
        nc.sync.dma_start(We3_sb[:], We3_d[:])

        if with_bias:
            b_row = []
            bsd_row = []
            for li in range(3):
                braw = spool.tile([1, NRC], f32, tag="braw")
                nc.sync.dma_start(braw[:], b_d[li][:])
                br = const.tile([1, NRC], f32r, tag=f"brow{li}")
                nc.vector.tensor_copy(br[:], braw[:])
                b_row.append(br)
                bsraw = spool.tile([1, NB], f32, tag="bsraw")
                nc.sync.dma_start(bsraw[:], bsd_d[li][:])
                bsr = const.tile([1, NB], f32r, tag=f"bsdrow{li}")
                nc.vector.tensor_copy(bsr[:], bsraw[:])
                bsd_row.append(bsr)
            beraw = spool.tile([1, H1], f32, tag="beraw")
            nc.sync.dma_start(beraw[:], be1_d[:])
            be1_row = const.tile([1, H1], f32r)
            nc.vector.tensor_copy(be1_row[:], beraw[:])
            be2_row = const.tile([1, H2], f32)
            nc.sync.dma_start(be2_row[:], be2_d[:])
            be3_row = const.tile([1, 1], f32)
            nc.sync.dma_start(be3_row[:], be3_d[:])

        # MLP lhsT staging: z^T chunks [128, kb, g]; kb 0..7 = x, 8 = mean, 9 = max
        zT = const.tile([128, 10, ng], f32r)

        # ---------------- per graph ----------------
        for _rep in range(repeat):
         for g in range(ng):
            # -- bonds (pre-transposed on host) -> multiplicative bf16 mask --
            bT_sb = gpool.tile([128, NB, N], i32, tag="bondsT")
            nc.sync.dma_start(bT_sb[:], bondsT_d[g])
            Mk = gpool.tile([128, NB, N], bf16, tag="mask")
            nc.gpsimd.tensor_copy(
                Mk[:].rearrange("p a b -> p (a b)"),
                bT_sb[:].rearrange("p a b -> p (a b)"),
            )

            # per-graph aug tiles: dstP p0=dst/p1=ones, srcP p0=ones/p1=src
            dstP_g = []
            srcP_g = []
            for db in range(2):
                dP = gpool.tile([2, R, 256], f32r, tag=f"dstp{db}")
                sP = gpool.tile([2, R, 256], f32r, tag=f"srcp{db}")
                nc.gpsimd.memset(dP[:].bitcast(f32), 1.0)
                nc.gpsimd.memset(sP[:].bitcast(f32), 1.0)
                dstP_g.append(dP)
                srcP_g.append(sP)

            # -- atoms transpose --
            at_nat = spool.tile([128, 2, CIN], f32, tag="atnat")
            for ib in range(2):
                nc.sync.dma_start(at_nat[:, ib, :], atoms_d[g, ib * 128:(ib + 1) * 128, :])
            atT_ps = ps_sm.tile([CIN, 2, 128], f32, tag="sm")
            for ib in range(2):
                nc.tensor.matmul(
                    atT_ps[:, ib, :], at_nat[:, ib, :], ident[:],
                    is_transpose=True, start=True, stop=True,
                )
            atoms_cur = gpool.tile([CIN, 256], f32r, tag="atoms0")
            nc.vector.tensor_copy(
                atoms_cur[:], atT_ps.rearrange("c a b -> c (a b)")
            )

            # -- x staging for MLP --
            x_stage = spool.tile([128, 8], f32, tag="xstage")
            nc.sync.dma_start(x_stage[:], x_d[g].rearrange("(f p) -> p f", p=128))
            nc.vector.tensor_copy(zT[:, 0:8, g:g + 1].rearrange("p a b -> p (a b)"),
                                  x_stage[:])

            # ---------------- GAT layers ----------------
            for li in range(3):
                W = W_sb[li]
                # h = atoms @ W (+b): out [i, (r,c)]; evicted to bf16
                h_bf = gpool3.tile([128, 2, NRC], bf16, tag="h")
                for ib in range(2):
                    hA = ps_sm.tile([128, 384], f32, tag="sm")
                    hB = ps_sm.tile([128, 256], f32, tag="sm")
                    lt = atoms_cur[:, ib * 128:(ib + 1) * 128]
                    mm(hA[:], lt, W[:, 0:384], start=True, stop=not with_bias)
                    mm(hB[:], lt, W[:, 384:NRC], start=True, stop=not with_bias)
                    if with_bias:
                        mm(hA[:], ones_row[:, :128], b_row[li][:, 0:384],
                           start=False, stop=True)
                        mm(hB[:], ones_row[:, :128], b_row[li][:, 384:NRC],
                           start=False, stop=True)
                    if ib == 0:
                        nc.vector.tensor_copy(h_bf[:, ib, 0:384], hA[:])
                        nc.scalar.activation(h_bf[:, ib, 384:NRC], hB[:], AF.Copy)
                    else:
                        nc.scalar.activation(h_bf[:, ib, 0:384], hA[:], AF.Copy)
                        nc.vector.tensor_copy(h_bf[:, ib, 384:NRC], hB[:])

                # src/dst logits straight from atomsT: sd = Wsd.T @ atomsT
                # rows 0-4 = src_r, rows 5-9 = dst_r
                sd_ps = ps_sm.tile([NB, 256], f32, tag="sm")
                mm(sd_ps[:], Wsd_sb[li][:], atoms_cur[:],
                   start=True, stop=not with_bias)
                if with_bias:
                    mm(sd_ps[:], bsd_row[li][:], ones_row[:],
                       start=False, stop=True)
                sd_sb = spool.tile([NB, 256], f32r, tag="sdsb")
                nc.vector.tensor_copy(sd_sb[:], sd_ps[:])
                # gather into the per-graph aug tiles (ones rows pre-set)
                dstP = dstP_g[li % 2]
                srcP = srcP_g[li % 2]
                nc.sync.dma_start(dstP[0:1], sd_sb[R:NB, :])
                nc.sync.dma_start(srcP[1:2], sd_sb[0:R, :])

                # S blocks -> prelu -> exp -> mask, in chunks of (4, 4, 2)
                Et = gpool3.tile([128, NB, 256], bf16, tag="et")
                b0 = 0
                for nb in (4, 4, 2):
                    L_sb = gpool3.tile([128, 4, 256], f32, tag="lsb")
                    Eu = gpool3.tile([128, 4, 256], bf16, tag="eu")
                    S_ps = ps_s.tile([128, 4, 256], f32, tag="sps")
                    for k in range(nb):
                        b = b0 + k
                        r, jh = b // 2, b % 2
                        mm(S_ps[:, k, :],
                           dstP[:, r, jh * 128:(jh + 1) * 128],
                           srcP[:, r, :],
                           start=True, stop=True)
                    # L = prelu(S) on ACT
                    nc.scalar.activation(
                        L_sb[:, 0:nb].rearrange("p a b -> p (a b)"),
                        S_ps[:, 0:nb].rearrange("p a b -> p (a b)"),
                        AF.Prelu, alpha=0.2,
                    )
                    # Eu = exp(L) -> bf16
                    nc.scalar.activation(
                        Eu[:, 0:nb].rearrange("p a b -> p (a b)"),
                        L_sb[:, 0:nb].rearrange("p a b -> p (a b)"),
                        AF.Exp,
                    )
                    # Et = Eu * mask  (bf16, DVE 2x)
                    nc.vector.tensor_tensor(
                        Et[:, b0:b0 + nb].rearrange("p a b -> p (a b)"),
                        Eu[:, 0:nb].rearrange("p a b -> p (a b)"),
                        Mk[:, b0:b0 + nb].rearrange("p a b -> p (a b)"),
                        op=OP.mult,
                    )
                    b0 += nb

                # aggregation out^T = sum_b h_b^T @ Et_b ; Z = sum_b ones^T @ Et_b
                o_ps = ps_sm.tile([128, 256], f32, tag="sm")
                for b in range(NB):
                    r, jh = b // 2, b % 2
                    mm(o_ps[:], h_bf[:, jh, r * 128:(r + 1) * 128],
                       Et[:, b, :], start=(b == 0), stop=(b == NB - 1))
                z_ps = ps_sm.tile([1, 256], f32, tag="sm")
                for b in range(NB):
                    mm(z_ps[:], ones_bf[:],
                       Et[:, b, :], start=(b == 0), stop=(b == NB - 1))

                # normalize (+ inter-layer leaky)
                rz_sb = spool.tile([1, 256], f32r, tag="rz")
                with nc.allow_low_precision(reason="f32r recip, 2^-12 rounding ok"):
                    nc.vector.reciprocal(rz_sb[:], z_ps[:])
                rzb_ps = ps_sm.tile([128, 256], f32, tag="sm")
                mm(rzb_ps[:], ones_row[:, :128], rz_sb[:], start=True, stop=True)
                O_sb = spool.tile([128, 256], f32, tag="osb")
                if li < 2:
                    nc.scalar.activation(O_sb[:], o_ps[:], AF.Prelu, alpha=0.2)
                else:
                    nc.scalar.activation(O_sb[:], o_ps[:], AF.Copy)
                nxt = gpool.tile([C, 256], f32r, tag=f"atoms{li + 1}")
                nc.vector.tensor_tensor(nxt[:], O_sb[:], rzb_ps[:], op=OP.mult)
                atoms_cur = nxt

            # y_feats: mean/max over atoms (free dim of h3T [c, i])
            h3T = atoms_cur
            mean_raw = spool.tile([128, 1], f32, tag="mean")
            nc.vector.tensor_reduce(mean_raw[:], h3T[:], axis=mybir.AxisListType.X,
                                    op=OP.add)
            nc.vector.tensor_scalar(zT[:, 8, g:g + 1], mean_raw[:], 1.0 / N, None,
                                    op0=OP.mult)
            nc.vector.tensor_reduce(zT[:, 9, g:g + 1], h3T[:], axis=mybir.AxisListType.X,
                                    op=OP.max)

         # ---------------- MLP head (batched over graphs) ---------------
         zz_ps = ps_sm.tile([ng, H1], f32, tag="sm")
         for kb in range(10):
            mm(zz_ps[:], zT[:, kb, :], We1_sb[:, kb, :],
               start=(kb == 0), stop=(kb == 9) and not with_bias)
         if with_bias:
            mm(zz_ps[:], ones_row[:, :ng], be1_row[:], start=False, stop=True)
         zzl = spool.tile([ng, H1], f32, tag="zzl")
         nc.scalar.activation(zzl[:], zz_ps[:], AF.Prelu, alpha=0.2)
         zzT_ps = ps_sm.tile([128, 2, ng], f32, tag="sm")
         for hh in range(2):
            nc.tensor.matmul(zzT_ps[:, hh, :], zzl[:, hh * 128:(hh + 1) * 128],
                             ident[:ng, :ng], is_transpose=True,
                             start=True, stop=True)
         zzT_sb = spool.tile([128, 2, ng], f32, tag="zzt")
         nc.vector.tensor_copy(zzT_sb[:], zzT_ps[:])

         z2_ps = ps_sm.tile([ng, H2], f32, tag="sm")
         for hh in range(2):
            nc.tensor.matmul(z2_ps[:], zzT_sb[:, hh, :], We2_sb[:, hh, :],
                             start=(hh == 0), stop=(hh == 1) and not with_bias)
         if with_bias:
            nc.tensor.matmul(z2_ps[:], onesrf[:, :ng], be2_row[:],
                             start=False, stop=True)
         z2l = spool.tile([ng, H2], f32, tag="z2l")
         nc.scalar.activation(z2l[:], z2_ps[:], AF.Prelu, alpha=0.2)
         z2T_ps = ps_sm.tile([H2, ng], f32, tag="sm")
         nc.tensor.matmul(z2T_ps[:], z2l[:], ident[:ng, :ng], is_transpose=True,
                         start=True, stop=True)
         z2T_sb = spool.tile([H2, ng], f32, tag="z2t")
         nc.vector.tensor_copy(z2T_sb[:], z2T_ps[:])

         y_ps = ps_sm.tile([ng, 1], f32, tag="sm")
         nc.tensor.matmul(y_ps[:], z2T_sb[:], We3_sb[:], start=True,
                         stop=not with_bias)
         if with_bias:
            nc.tensor.matmul(y_ps[:], onesrf[:, :ng], be3_row[:],
                             start=False, stop=True)
         y_sb = spool.tile([ng, 1], f32, tag="y")
         nc.vector.tensor_copy(y_sb[:], y_ps[:])
         nc.sync.dma_start(out_d[:], y_sb[:])

    nc.compile()
    _BUILD_CACHE[key] = nc
    return nc


_PARAM_KEYS = ("W1", "W2", "W3", "We1", "We2", "We3")
_BIAS_KEYS = ("b1", "b2", "b3", "be1", "be2", "be3")


def _fold_sd(W, a, b):
    """Wsd[k, 0:5] = sum_c W[k,(r,c)] a[r,c]; [:, 5:10] dst half. bsd likewise."""
    W = np.asarray(W, np.float64)
    a = np.asarray(a, np.float64)
    cin = W.shape[0]
    Wr = W.reshape(cin, R, C)
    asrc, adst = a[:, :C], a[:, C:]
    Wsrc = np.einsum("krc,rc->kr", Wr, asrc)
    Wdst = np.einsum("krc,rc->kr", Wr, adst)
    Wsd = np.concatenate([Wsrc, Wdst], axis=1).astype(np.float32)
    br = np.asarray(b, np.float64).reshape(R, C)
    bsd = np.concatenate(
        [np.einsum("rc,rc->r", br, asrc), np.einsum("rc,rc->r", br, adst)]
    ).reshape(1, NB).astype(np.float32)
    return np.ascontiguousarray(Wsd), np.ascontiguousarray(bsd)


def _transpose_bonds(yb):
    """[ng, i, j, r] i32 -> [ng, j', b=2r+jh, i] i32 (j = jh*128 + j')."""
    ng = yb.shape[0]
    bt = yb.transpose(0, 2, 3, 1)            # [ng, j, r, i]
    bt = bt.reshape(ng, 2, 128, R, N)        # [ng, jh, j', r, i]
    bt = bt.transpose(0, 2, 3, 1, 4)         # [ng, j', r, jh, i]
    return np.ascontiguousarray(bt.reshape(ng, 128, NB, N), np.int32)


def _shard_inputs(inputs, with_bias, n_cores, ng):
    wsd = {}
    for i in (1, 2, 3):
        wsd[f"Wsd{i}"], wsd[f"bsd{i}"] = _fold_sd(
            inputs[f"W{i}"], inputs[f"a{i}"], inputs[f"b{i}"]
        )
    per_core = []
    for c in range(n_cores):
        s = slice(c * ng, (c + 1) * ng)
        m = {
            "y_atoms": np.ascontiguousarray(inputs["y_atoms"][s], np.float32),
            "bonds_t": _transpose_bonds(np.asarray(inputs["y_bonds"][s], np.int32)),
            "x": np.ascontiguousarray(inputs["x"][s], np.float32),
        }
        for k in _PARAM_KEYS:
            m[k] = np.ascontiguousarray(inputs[k], np.float32)
        for i in (1, 2, 3):
            m[f"Wsd{i}"] = wsd[f"Wsd{i}"]
        if with_bias:
            for k in _BIAS_KEYS:
                m[k] = np.ascontiguousarray(np.asarray(inputs[k], np.float32).reshape(1, -1))
            for i in (1, 2, 3):
                m[f"bsd{i}"] = wsd[f"bsd{i}"]
        per_core.append(m)
    return per_core


def _needs_bias(inputs):
    return any(np.abs(np.asarray(inputs[k])).max() > 0 for k in _BIAS_KEYS)


def kernel(**inputs):
    from concourse.bass_utils import run_bass_kernel_spmd

    with_bias = _needs_bias(inputs)
    nc = build(NG, with_bias)
    in_maps = _shard_inputs(inputs, with_bias, NCORE, NG)
    res = run_bass_kernel_spmd(nc, in_maps, core_ids=list(range(NCORE)))
    out = np.concatenate([r["out"] for r in res.results], axis=0)
    return np.ascontiguousarray(out, np.float32)
